# revision 5
# baseline (speedup 1.0000x reference)
# MoE (top-2 of 8 experts) kernel for 8 Trainium2 NeuronCores.
#
# Strategy: expert-parallel sparse routing. The reference computes every
# expert densely, but the output only depends on each token's top-2
# experts. Host computes the (tiny) gating network and per-expert token
# lists; core e runs expert e's FFN (x@W1+b1 -> LayerNorm -> erf-GELU ->
# @W2+b2) on just its routed tokens; host applies the gate weights in
# the combine. Matmuls run in float32r (TF32) at full PE rate, tokens on
# the moving (free) dimension for both matmuls so no on-device
# transposes are needed. LayerNorm reductions over H (the partition dim)
# are done with ones-vector matmuls on the PE; per-token stats are
# broadcast back across partitions with K=1 matmuls.

import numpy as np

import concourse.bacc as bacc
import concourse.mybir as mybir
import concourse.tile as tile
from concourse.bass_utils import run_bass_kernel_spmd

P = 128
D, H, E, TOPK = 1024, 2048, 8, 2
DS, NJ, NK = D // P, H // P, H // P
LN_EPS = 1e-5
TT = 256          # token tile (psum moving free dim; >=256 keeps fp32r at 1 cyc/row)
G = 2             # t-tiles per W1-stream group (W1 reloaded once per G*TT tokens)

_kernel_cache: dict[int, object] = {}


def _build(C: int):
    f32, f32r = mybir.dt.float32, mybir.dt.float32r
    nc = bacc.Bacc("TRN2", target_bir_lowering=False, debug=False, num_devices=8)
    xT = nc.dram_tensor("xT", [P, DS, C], f32r, kind="ExternalInput").ap()
    W1 = nc.dram_tensor("W1", [NJ, P, DS, P], f32r, kind="ExternalInput").ap()
    W2 = nc.dram_tensor("W2", [P, NJ, H], f32r, kind="ExternalInput").ap()
    b1 = nc.dram_tensor("b1", [P, NJ], f32, kind="ExternalInput").ap()
    lg = nc.dram_tensor("lg", [P, NJ], f32, kind="ExternalInput").ap()
    lb = nc.dram_tensor("lb", [P, NJ], f32, kind="ExternalInput").ap()
    b2 = nc.dram_tensor("b2", [P, NK], f32, kind="ExternalInput").ap()
    outT = nc.dram_tensor("outT", [NK, P, C], f32, kind="ExternalOutput").ap()

    n_pairs = C // (TT * G)
    Gelu = mybir.ActivationFunctionType.Gelu
    Ident = mybir.ActivationFunctionType.Identity
    Sqrt = mybir.ActivationFunctionType.Sqrt

    with tile.TileContext(nc) as tc:
        with (
            tc.tile_pool(name="const", bufs=1) as constp,
            tc.tile_pool(name="w2p", bufs=1) as w2p,
            tc.tile_pool(name="w1p", bufs=3) as w1p,
            tc.tile_pool(name="xp", bufs=2) as xp,
            tc.tile_pool(name="hp", bufs=2) as hp,
            tc.tile_pool(name="sqp", bufs=2) as sqp,
            tc.tile_pool(name="op", bufs=3) as op,
            tc.tile_pool(name="statp", bufs=1) as statp,
            tc.tile_pool(name="ps_mm", bufs=2, space="PSUM") as ps_mm,
            tc.tile_pool(name="ps_acc", bufs=2, space="PSUM") as ps_acc,
            tc.tile_pool(name="ps_bc", bufs=1, space="PSUM") as ps_bc,
        ):
            b1s = constp.tile([P, NJ], f32)
            nc.sync.dma_start(b1s[:], b1[:])
            lgs = constp.tile([P, NJ], f32)
            nc.sync.dma_start(lgs[:], lg[:])
            lbs = constp.tile([P, NJ], f32)
            nc.sync.dma_start(lbs[:], lb[:])
            b2s = constp.tile([P, NK], f32)
            nc.sync.dma_start(b2s[:], b2[:])
            ones_f = constp.tile([P, 1], f32)
            nc.any.memset(ones_f[:], 1.0)
            ones_c = constp.tile([P, 1], f32r)  # lhsT for partition-sum matmuls
            nc.vector.tensor_copy(ones_c[:], ones_f[:])
            oner_f = constp.tile([1, P], f32)
            nc.any.memset(oner_f[:], 1.0)
            oner_c = constp.tile([1, P], f32r)  # lhsT for partition-broadcast matmuls
            nc.vector.tensor_copy(oner_c[:], oner_f[:])
            eps_t = constp.tile([1, 1], f32)
            nc.any.memset(eps_t[:], LN_EPS)

            # Cache all of W2 in SBUF (16 MB, f32r) in 4 DMA chunks.
            w2sb = w2p.tile([P, NJ, H], f32r)
            for c4 in range(4):
                nc.sync.dma_start(
                    w2sb[:, 4 * c4 : 4 * (c4 + 1), :], W2[:, 4 * c4 : 4 * (c4 + 1), :]
                )

            for pair in range(n_pairs):
                xts = []
                for g in range(G):
                    t0 = (pair * G + g) * TT
                    xt = xp.tile([P, DS, TT], f32r, tag="xt")
                    nc.sync.dma_start(xt[:], xT[:, :, t0 : t0 + TT])
                    xts.append(xt)

                hs = [hp.tile([P, NJ, TT], f32r, tag="h", name="h") for _ in range(G)]
                s_ps = [
                    ps_acc.tile([1, TT], f32, tag="sacc", name="sacc") for _ in range(G)
                ]
                q_ps = [
                    ps_acc.tile([1, TT], f32, tag="qacc", name="qacc") for _ in range(G)
                ]

                # ---- mm1 + LN statistics ----
                for j in range(NJ):
                    w1t = w1p.tile([P, DS, P], f32r, tag="w1")
                    nc.sync.dma_start(w1t[:], W1[j])
                    for g in range(G):
                        pm = ps_mm.tile([P, TT], f32, tag="mm")
                        for ds in range(DS):
                            nc.tensor.matmul(
                                pm[:],
                                w1t[:, ds, :],
                                xts[g][:, ds, :],
                                start=(ds == 0),
                                stop=(ds == DS - 1),
                            )
                        # evict psum -> h (f32r) with per-partition bias b1[j]
                        nc.scalar.activation(
                            hs[g][:, j, :], pm[:], Ident, bias=b1s[:, j : j + 1]
                        )
                        sq = sqp.tile([P, TT], f32r, tag="sq")
                        nc.vector.tensor_mul(sq[:], hs[g][:, j, :], hs[g][:, j, :])
                        nc.tensor.matmul(
                            s_ps[g][:],
                            ones_c[:],
                            hs[g][:, j, :],
                            start=(j == 0),
                            stop=(j == NJ - 1),
                        )
                        nc.tensor.matmul(
                            q_ps[g][:],
                            ones_c[:],
                            sq[:],
                            start=(j == 0),
                            stop=(j == NJ - 1),
                        )

                # ---- normalize + GELU + mm2, per t-tile in the pair ----
                for g in range(G):
                    mu = statp.tile([1, TT], f32, tag="mu")
                    nc.vector.tensor_scalar_mul(mu[:], s_ps[g][:], 1.0 / H)
                    msq = statp.tile([1, TT], f32, tag="msq")
                    nc.vector.tensor_scalar_mul(msq[:], q_ps[g][:], 1.0 / H)
                    mu2 = statp.tile([1, TT], f32, tag="mu2")
                    nc.vector.tensor_mul(mu2[:], mu[:], mu[:])
                    var = statp.tile([1, TT], f32, tag="var")
                    nc.vector.tensor_sub(var[:], msq[:], mu2[:])
                    std = statp.tile([1, TT], f32, tag="std")
                    nc.scalar.activation(std[:], var[:], Sqrt, bias=eps_t[:])
                    rstd = statp.tile([1, TT], f32, tag="rstd")
                    nc.vector.reciprocal(rstd[:], std[:])
                    # A = rstd, B = mu * rstd (f32r rows for broadcast matmuls)
                    a_row = statp.tile([1, TT], f32r, tag="a_row")
                    nc.vector.tensor_copy(a_row[:], rstd[:])
                    b_row = statp.tile([1, TT], f32r, tag="b_row")
                    nc.vector.tensor_mul(b_row[:], mu[:], rstd[:])
                    # broadcast across partitions via K=1 matmuls
                    a_bc = ps_bc.tile([P, TT], f32, tag="a_bc")
                    nc.tensor.matmul(a_bc[:], oner_c[:], a_row[:], start=True, stop=True)
                    b_bc = ps_bc.tile([P, TT], f32, tag="b_bc")
                    nc.tensor.matmul(b_bc[:], oner_c[:], b_row[:], start=True, stop=True)
                    for j in range(NJ):
                        hj = hs[g][:, j, :]
                        nc.vector.tensor_mul(hj, hj, a_bc[:])
                        nc.vector.tensor_sub(hj, hj, b_bc[:])
                        nc.scalar.activation(
                            hj, hj, Gelu, bias=lbs[:, j : j + 1], scale=lgs[:, j : j + 1]
                        )

                for g in range(G):
                    t0 = (pair * G + g) * TT
                    for k in range(NK):
                        pm = ps_mm.tile([P, TT], f32, tag="mm")
                        for js in range(NJ):
                            nc.tensor.matmul(
                                pm[:],
                                w2sb[:, js, k * P : (k + 1) * P],
                                hs[g][:, js, :],
                                start=(js == 0),
                                stop=(js == NJ - 1),
                            )
                        ot = op.tile([P, TT], f32, tag="out")
                        nc.scalar.activation(ot[:], pm[:], Ident, bias=b2s[:, k : k + 1])
                        nc.sync.dma_start(outT[k, :, t0 : t0 + TT], ot[:])

    nc.compile()
    return nc


def _route(x64, Wg64, bg64):
    """Host gating: returns per-token top-2 expert ids and renormalized weights."""
    logits = x64 @ Wg64 + bg64                      # [N, E] fp64
    order = np.argsort(-logits, axis=1, kind="stable")[:, :TOPK]
    l0 = np.take_along_axis(logits, order, axis=1)  # [N, 2] descending
    # pair softmax == softmax-then-renormalize over the top-2
    w0 = 1.0 / (1.0 + np.exp(l0[:, 1] - l0[:, 0]))
    w = np.stack([w0, 1.0 - w0], axis=1)
    return order, w


def kernel(x, W1, b1, ln_g, ln_b, W2, b2, Wg, bg):
    x = np.ascontiguousarray(np.asarray(x, dtype=np.float32))
    W1 = np.asarray(W1, dtype=np.float32)
    b1 = np.asarray(b1, dtype=np.float32)
    ln_g = np.asarray(ln_g, dtype=np.float32)
    ln_b = np.asarray(ln_b, dtype=np.float32)
    W2 = np.asarray(W2, dtype=np.float32)
    b2 = np.asarray(b2, dtype=np.float32)
    Wg = np.asarray(Wg, dtype=np.float32)
    bg = np.asarray(bg, dtype=np.float32)
    N = x.shape[0]

    order, w = _route(x.astype(np.float64), Wg.astype(np.float64), bg.astype(np.float64))

    # Per-expert token lists, padded to a common capacity C (multiple of TT*G).
    tok_idx, tok_w = [], []
    for e in range(E):
        sel = np.nonzero((order[:, 0] == e) | (order[:, 1] == e))[0]
        we = np.where(order[sel, 0] == e, w[sel, 0], w[sel, 1]).astype(np.float32)
        tok_idx.append(sel)
        tok_w.append(we)
    step = TT * G
    C = max(step, int(-(-max(len(s) for s in tok_idx) // step)) * step)

    if C not in _kernel_cache:
        _kernel_cache[C] = _build(C)
    nc = _kernel_cache[C]

    in_maps = []
    for e in range(E):
        idx = np.zeros(C, dtype=np.int64)
        idx[: len(tok_idx[e])] = tok_idx[e]
        xg = x[idx]                                   # [C, D]
        xT_dev = np.ascontiguousarray(xg.reshape(C, DS, P).transpose(2, 1, 0))
        W1_dev = np.ascontiguousarray(
            W1[e].reshape(DS, P, NJ, P).transpose(2, 1, 0, 3)
        )
        W2_dev = np.ascontiguousarray(W2[e].reshape(NJ, P, H).transpose(1, 0, 2))
        in_maps.append(
            {
                "xT": xT_dev,
                "W1": W1_dev,
                "W2": W2_dev,
                "b1": np.ascontiguousarray(b1[e].reshape(NJ, P).T),
                "lg": np.ascontiguousarray(ln_g[e].reshape(NJ, P).T),
                "lb": np.ascontiguousarray(ln_b[e].reshape(NJ, P).T),
                "b2": np.ascontiguousarray(b2[e].reshape(NK, P).T),
            }
        )

    res = run_bass_kernel_spmd(nc, in_maps, core_ids=list(range(E)))

    y = np.zeros((N, H), dtype=np.float32)
    for e in range(E):
        cnt = len(tok_idx[e])
        eoT = res.results[e]["outT"].reshape(H, C)
        y[tok_idx[e]] += tok_w[e][:, None] * eoT[:, :cnt].T
    return y
